# revision 1
# baseline (speedup 1.0000x reference)
"""Trainium2 Bass kernel for nn_CrossAttention (B=8, N=4096, S=512, D=512, H=8).

Sharding: data-parallel over batch — each of the 8 NeuronCores computes the
full cross-attention for one batch element. No collectives needed.

Per-core dataflow (all activations kept feature-major so no on-chip
transposes are ever required):
  - host pre-transposes x[b] -> xT [D, N] and context[b] -> ctxT [D, S]
  - qT[o, n]  = sum_i wqT[i, o] * xT[i, n]          (PE)
  - kT[dk, s] = sum_i wkT[i, dk] * ctxT[i, s]       (PE)
  - v[s, dv]  = sum_i ctxT[i, s] * wvT[i, dv]       (PE, token-major v)
    v is stored interleaved with a ones column per head: vext[s, h, 0:64]=v,
    vext[s, h, 64]=1 so the attention matmul also produces the softmax
    denominator for free (M=65).
  - scoresT[s, n] = kT_h.T @ qT_h per head          (PE, K=64, heads paired
    into PE row groups 0-63 / 64-127 for concurrency)
  - e = exp(SCALE*scoresT + amask_bias)             (ACT; mask folded into a
    per-partition bias so masked rows give exp(-30000)=0; no max-subtraction
    needed since |SCALE*scores| is O(1) for this problem scale)
  - OtildeT'[{d,den}, n] = vext_h.T @ e_h           (PE, K=128, M=65)
  - rden = 1/den  (DVE), broadcast across partitions via a DRAM bounce
  - OT = OtildeT * R                                (DVE)
  - y[n, o] = sum_c OT[c, n-slice].T @ wpT[c, o]    (PE, token-major output,
    so the DMA store to DRAM is contiguous)

Matmul inputs are kept in MMDT (float32r or bfloat16, env KMMDT to override);
accumulation is always fp32 in PSUM and the softmax/normalization runs fp32.
"""

import os

import numpy as np

try:
    import concourse.bass as bass
except ImportError:
    import sys

    sys.path.insert(0, "/opt/trn_rl_repo")
    import concourse.bass as bass

from contextlib import ExitStack

import concourse.mybir as mybir
import concourse.tile as tile
from concourse.bass import ts

B, N, S, D, H = 8, 4096, 512, 512, 8
HD = D // H  # 64
SCALE = HD**-0.5
P = 128
IC = D // P  # 4 chunks of the contraction/feature dims
SC = S // P  # 4 chunks of the context length
NT = 512  # queries per outer tile
NTILES = N // NT  # 8
NSUB = NT // P  # 4
MASK_NEG = -30000.0

f32 = mybir.dt.float32

MMDT_NAME = os.environ.get("KMMDT", "float32r")


def _np_mm(mmdt):
    return np.dtype(mybir.dt.np(mmdt))


def _split_multi_waits(nc: bass.Bass) -> None:
    """This walrus toolchain accepts at most ONE sync-wait per instruction
    ("Too many sync wait commands" in setupSyncWait, seen for MM/LW, NoOp,
    and DMA structs alike). Hoist all but the last wait of any instruction
    onto a chain of same-engine InstNoOps spliced immediately before it —
    same program position, so synchronization semantics are unchanged."""
    eng_map = {
        mybir.EngineType.PE: lambda: nc.tensor,
        mybir.EngineType.Activation: lambda: nc.scalar,
        mybir.EngineType.DVE: lambda: nc.vector,
        mybir.EngineType.Pool: lambda: nc.gpsimd,
        mybir.EngineType.SP: lambda: nc.sync,
    }
    for fn in nc.m.functions:
        blocks = fn.blocks
        for bb in blocks:
            insts = list(bb.instructions)
            out = []
            changed = False
            for inst in insts:
                si = inst.sync_info
                if (
                    si is not None
                    and len(si.on_wait) > 1
                    and inst.engine in eng_map
                ):
                    waits = list(si.on_wait)
                    for w in waits[:-1]:  # one nop per excess wait
                        nop = eng_map[inst.engine]().nop(nofuse=True).ins
                        # the nop was appended to whatever block is current;
                        # strip it from there before splicing it in place
                        for bb2 in blocks:
                            lst = list(bb2.instructions)
                            if any(x.name == nop.name for x in lst):
                                bb2.instructions = [
                                    x for x in lst if x.name != nop.name
                                ]
                                if bb2 is bb:
                                    insts = [
                                        x for x in insts if x.name != nop.name
                                    ]
                        nop.sync_info = mybir.SyncInfo(
                            on_wait=[w], on_update=[]
                        )
                        out.append(nop)
                    inst.sync_info = mybir.SyncInfo(
                        on_wait=waits[-1:], on_update=list(si.on_update)
                    )
                    changed = True
                out.append(inst)
            if changed:
                bb.instructions = out


def _build_nc(mmdt_name: str, has_bq, has_bk, has_bv, has_bp) -> bass.Bass:
    mmdt = getattr(mybir.dt, mmdt_name)
    nc = bass.Bass()

    xT = nc.dram_tensor("xT", [D, N], mmdt, kind="ExternalInput")
    ctxT = nc.dram_tensor("ctxT", [D, S], mmdt, kind="ExternalInput")
    wqT = nc.dram_tensor("wqT", [D, D], mmdt, kind="ExternalInput")
    wkT = nc.dram_tensor("wkT", [D, D], mmdt, kind="ExternalInput")
    wvT = nc.dram_tensor("wvT", [D, D], mmdt, kind="ExternalInput")
    wpT = nc.dram_tensor("wpT", [D, D], mmdt, kind="ExternalInput")
    bq = nc.dram_tensor("bq", [D, 1], f32, kind="ExternalInput")
    bk = nc.dram_tensor("bk", [D, 1], f32, kind="ExternalInput")
    bv = nc.dram_tensor("bv", [1, D], mmdt, kind="ExternalInput")
    bp = nc.dram_tensor("bp", [1, D], mmdt, kind="ExternalInput")
    amask = nc.dram_tensor("amask", [S, 1], f32, kind="ExternalInput")
    y = nc.dram_tensor("y", [N, D], f32, kind="ExternalOutput")

    rden_dram = nc.dram_tensor("rden_scratch", [NTILES, H, NT], f32)

    ch = lambda dram: dram.rearrange("(c p) o -> p c o", p=P)  # [P, IC, D]

    with tile.TileContext(nc) as tc, ExitStack() as ctx:
        const = ctx.enter_context(tc.tile_pool(name="const", bufs=1))
        work = ctx.enter_context(tc.tile_pool(name="work", bufs=2))
        epool = ctx.enter_context(tc.tile_pool(name="epool", bufs=12))
        ypool = ctx.enter_context(tc.tile_pool(name="ypool", bufs=4))
        psum = ctx.enter_context(tc.tile_pool(name="psum", bufs=1, space="PSUM"))

        # ---- persistent tiles -------------------------------------------
        wq_t = const.tile([P, IC, D], mmdt)
        wk_t = const.tile([P, IC, D], mmdt)
        wv_t = const.tile([P, IC, D], mmdt)
        wp_t = const.tile([P, IC, D], mmdt)
        ctx_t = const.tile([P, IC, S], mmdt)
        amask_t = const.tile([P, SC, 1], f32)
        nc.sync.dma_start(wq_t[:], ch(wqT))
        nc.sync.dma_start(wk_t[:], ch(wkT))
        nc.sync.dma_start(wv_t[:], ch(wvT))
        nc.sync.dma_start(wp_t[:], ch(wpT))
        nc.sync.dma_start(ctx_t[:], ch(ctxT))
        nc.sync.dma_start(amask_t[:], amask.rearrange("(c p) o -> p c o", p=P))

        if has_bq:
            bq_t = const.tile([P, IC, 1], f32)
            nc.sync.dma_start(bq_t[:], bq.rearrange("(c p) o -> p c o", p=P))
        if has_bk:
            bk_t = const.tile([P, IC, 1], f32)
            nc.sync.dma_start(bk_t[:], bk.rearrange("(c p) o -> p c o", p=P))
        if has_bv or has_bp:
            ones1_t = const.tile([1, P], mmdt)
            nc.vector.memset(ones1_t[:], 1.0)
        if has_bv:
            bv_t = const.tile([1, D], mmdt)
            nc.sync.dma_start(bv_t[:], bv[:])
        if has_bp:
            bp_t = const.tile([1, D], mmdt)
            nc.sync.dma_start(bp_t[:], bp[:])

        kT_t = const.tile([P, IC, S], mmdt)  # feature-major keys
        vext_t = const.tile([P, SC, H, HD + 1], mmdt)  # token-major v + ones col

        # ---- kv projections (once per core) -----------------------------
        ones_cast = f32 if mmdt_name == "float32r" else mmdt
        for sc in range(SC):
            for h in range(H):
                nc.vector.memset(vext_t[:, sc, h, HD : HD + 1].bitcast(ones_cast), 1.0)

        for kc in range(IC):  # dk chunks -> kT
            ps = psum.tile([P, S], f32, tag="ps_q", bufs=2)
            for i in range(IC):
                nc.tensor.matmul(
                    ps[:],
                    wk_t[:, i, ts(kc, P)],
                    ctx_t[:, i, :],
                    start=(i == 0),
                    stop=(i == IC - 1),
                )
            if has_bk:
                nc.vector.tensor_scalar_add(kT_t[:, kc, :], ps[:], bk_t[:, kc, :])
            else:
                nc.vector.tensor_copy(kT_t[:, kc, :], ps[:])

        for sc in range(SC):  # s chunks -> v (token-major)
            ps = psum.tile([P, D], f32, tag="ps_o", bufs=2)
            for i in range(IC):
                nc.tensor.matmul(
                    ps[:],
                    ctx_t[:, i, ts(sc, P)],
                    wv_t[:, i, :],
                    start=(i == 0),
                    stop=(i == IC - 1 and not has_bv),
                )
            if has_bv:
                nc.tensor.matmul(ps[:], ones1_t[:], bv_t[:], start=False, stop=True)
            for h in range(H):
                nc.vector.tensor_copy(
                    vext_t[:, sc, h, 0:HD], ps[:, h * HD : (h + 1) * HD]
                )

        # ---- main loop over query tiles ---------------------------------
        for t in range(NTILES):
            xT_t = work.tile([P, IC, NT], mmdt, tag="xT")
            nc.sync.dma_start(
                xT_t[:], xT[:, ts(t, NT)].rearrange("(c p) n -> p c n", p=P)
            )

            # qT for this tile (feature-major)
            qT_t = work.tile([P, IC, NT], mmdt, tag="qT")
            for oc in range(IC):
                ps = psum.tile([P, NT], f32, tag="ps_q", bufs=2)
                for i in range(IC):
                    nc.tensor.matmul(
                        ps[:],
                        wq_t[:, i, ts(oc, P)],
                        xT_t[:, i, :],
                        start=(i == 0),
                        stop=(i == IC - 1),
                    )
                if has_bq:
                    nc.vector.tensor_scalar_add(qT_t[:, oc, :], ps[:], bq_t[:, oc, :])
                else:
                    nc.vector.tensor_copy(qT_t[:, oc, :], ps[:])

            oexts = {}
            for c in range(IC):  # head pair (2c, 2c+1); kT/qT chunk c
                es = {0: [], 1: []}
                for sc in range(SC):
                    for par in (0, 1):  # PE row groups 0-63 / 64-127
                        pslc = slice(par * HD, (par + 1) * HD)
                        ps_s = psum.tile([P, NT], f32, tag="ps_s", bufs=3)
                        nc.tensor.matmul(
                            ps_s[:],
                            kT_t[pslc, c, ts(sc, P)],
                            qT_t[pslc, c, :],
                            start=True,
                            stop=True,
                        )
                        e = epool.tile([P, NT], mmdt, tag="e")
                        nc.scalar.activation(
                            e[:],
                            ps_s[:],
                            mybir.ActivationFunctionType.Exp,
                            bias=amask_t[:, sc, :],
                            scale=SCALE,
                        )
                        es[par].append(e)

                for par in (0, 1):
                    h = 2 * c + par
                    ps_o = psum.tile([P, NT], f32, tag="ps_o", bufs=2)
                    for sc in range(SC):
                        nc.tensor.matmul(
                            ps_o[0 : HD + 1, :],
                            vext_t[:, sc, h, :],
                            es[par][sc][:],
                            start=(sc == 0),
                            stop=(sc == SC - 1),
                        )
                    # Otilde' (rows 0-63 = unnormalized out, row 64 = denom)
                    oe = epool.tile([HD + 1, NT], f32, tag="oext")
                    nc.vector.tensor_copy(oe[:], ps_o[0 : HD + 1, :])
                    oexts[h] = oe
                    # denominator row straight to DRAM for the broadcast bounce
                    nc.sync.dma_start(rden_dram[t, h : h + 1], oe[HD : HD + 1, :])

            # fetch denominators broadcast across 64 partitions per head,
            # reciprocal after broadcast, then normalize Otilde -> OT.
            # Compute ops need all operands at the same start partition, so the
            # odd head of each pair is staged into partitions 64:128 via an
            # SBUF->SBUF DMA (DMA has no partition-alignment restriction).
            ot_t = work.tile([P, IC, NT], mmdt, tag="ot")
            stag_t = work.tile([P, IC, NT], f32, tag="stag")
            den_t = work.tile([P, IC, NT], f32, tag="den")
            for c in range(IC):
                for par in (0, 1):
                    nc.sync.dma_start(
                        den_t[par * HD : (par + 1) * HD, c, :],
                        rden_dram[t, 2 * c + par : 2 * c + par + 1].to_broadcast(
                            (HD, NT)
                        ),
                    )
                nc.vector.reciprocal(den_t[:, c, :], den_t[:, c, :])
                nc.vector.tensor_mul(
                    ot_t[0:HD, c, :], oexts[2 * c][0:HD, :], den_t[0:HD, c, :]
                )
                nc.sync.dma_start(stag_t[HD:P, c, :], oexts[2 * c + 1][0:HD, :])
                nc.vector.tensor_mul(
                    ot_t[HD:P, c, :], stag_t[HD:P, c, :], den_t[HD:P, c, :]
                )

            # output projection, token-major
            for ns in range(NSUB):
                ps_y = psum.tile([P, D], f32, tag="ps_y", bufs=1)
                for c in range(IC):
                    nc.tensor.matmul(
                        ps_y[:],
                        ot_t[:, c, ts(ns, P)],
                        wp_t[:, c, :],
                        start=(c == 0),
                        stop=(c == IC - 1 and not has_bp),
                    )
                if has_bp:
                    nc.tensor.matmul(ps_y[:], ones1_t[:], bp_t[:], start=False, stop=True)
                y_t = ypool.tile([P, D], f32, tag="y")
                nc.vector.tensor_copy(y_t[:], ps_y[:])
                nc.sync.dma_start(y[t * NT + ns * P : t * NT + (ns + 1) * P, :], y_t[:])

    _split_multi_waits(nc)
    return nc


_NC_CACHE: dict = {}


def _get_nc(flags):
    if flags not in _NC_CACHE:
        _NC_CACHE[flags] = _build_nc(*flags)
    return _NC_CACHE[flags]


def _prep_in_maps(x, context, context_mask, wq, bq, wkv, bkv, wp, bp, mmdt_name=None):
    if mmdt_name is None:
        mmdt_name = MMDT_NAME
    np_mm = _np_mm(getattr(mybir.dt, mmdt_name))
    cvt = lambda a: np.ascontiguousarray(a).astype(np_mm, copy=False)
    wqT = cvt(wq.T)
    wkT = cvt(wkv[:D].T)
    wvT = cvt(wkv[D:].T)
    wpT = cvt(wp.T)
    bq_c = np.ascontiguousarray(bq.reshape(D, 1), dtype=np.float32)
    bk_c = np.ascontiguousarray(bkv[:D].reshape(D, 1), dtype=np.float32)
    bv_r = cvt(bkv[D:].reshape(1, D))
    bp_r = cvt(bp.reshape(1, D))
    flags = (
        mmdt_name,
        bool(np.any(bq != 0)),
        bool(np.any(bkv[:D] != 0)),
        bool(np.any(bkv[D:] != 0)),
        bool(np.any(bp != 0)),
    )
    in_maps = []
    for b in range(B):
        amask = np.where(context_mask[b], np.float32(MASK_NEG), np.float32(0.0))
        in_maps.append(
            {
                "xT": cvt(x[b].T),
                "ctxT": cvt(context[b].T),
                "wqT": wqT,
                "wkT": wkT,
                "wvT": wvT,
                "wpT": wpT,
                "bq": bq_c,
                "bk": bk_c,
                "bv": bv_r,
                "bp": bp_r,
                "amask": amask.astype(np.float32).reshape(S, 1),
            }
        )
    return in_maps, flags


def kernel(x, context, context_mask, wq, bq, wkv, bkv, wp, bp):
    from concourse.bass_utils import run_bass_kernel_spmd

    in_maps, flags = _prep_in_maps(
        x, context, context_mask, wq, bq, wkv, bkv, wp, bp
    )
    nc = _get_nc(flags)
    res = run_bass_kernel_spmd(nc, in_maps, list(range(B)))
    return np.stack([np.asarray(res.results[b]["y"]) for b in range(B)], axis=0)

